# revision 18
# baseline (speedup 1.0000x reference)
"""Sparse (sliding-window + sink) GQA attention block on 8 TRN2 NeuronCores.

v6: fp8e4 hi/lo-pair DoubleRow matmuls for the projections and out-proj
(2x PE throughput at ~bf16 accuracy), streamed x with per-pair DMA chunks,
weights-first two-queue schedule, single 8-bank PSUM budget.

Sharding: tensor-parallel over the 64 q-heads -> 8 q-heads (= 1 kv-head
group) per core; x replicated; wo partial outputs summed on host.

Per-core dataflow:
  A:  qkv projections in fp8 hi/lo pairs: value v ~= hi + lo (both e4m3),
      product (xh+xl)(wh+wl) ~= wh*xh + wh*xl + wl*xh via three DoubleRow
      matmuls per k-pair (weights scaled x16 on host; drain scale 1/16 and
      the real bias via the ACT activation bias operand).  Pass 1 streams
      x pairs (kv + q0); pass 2 (g1..g3) re-reads resident x.
  B:  RoPE rotate-half via a signed permutation matmul on the PE, then
      bf16 DVE mults; 0.125 score scale baked into the q-side tables;
      v transposed via the XBAR DMA transpose on the SP queue.
  C:  two half-passes (i<512, i>=512), software-pipelined across heads.
      Per head-half: scoresT into PSUM, exp on ACT, 0/1-mask multiply on
      GpSimd (bf16), attnT accumulated per i-block in PSUM; denominators
      via the v ones-row; reciprocal_approx_fast; per-pair broadcast via
      a selector matmul.  Attention output split to fp8 hi/lo on DVE.
  D:  out[i,dd] partials via fp8 DoubleRow over et-pairs (3 combos), per-it
      batched bf16 DMA out alternating queues.
"""

import numpy as np

B, S, DIM = 1, 1024, 2880
H, HKV, HD = 64, 8, 64
GROUP = H // HKV
WINDOW = 128
THETA = 150000.0
NC = 8
HL = H // NC                 # 8 local q-heads per core
EL = HL * HD                 # 512 local q-dim
DT2 = 24                     # fp8 k-tiles (22.5 rounded up to 12 pairs)
PR = DT2 // 2                # 12 k-pairs
NJ = S // 128                # 8 j/i blocks
DDC = 480                    # out-proj column chunk (6 per row-block)
SCW = 16.0                   # host-side weight scale (fp8 subnormal guard)

_cache = {}


def _build_module():
    import concourse.bacc as bacc
    import concourse.mybir as mybir
    import concourse.tile as tile

    f32 = mybir.dt.float32
    bf16 = mybir.dt.bfloat16
    f8 = mybir.dt.float8e4
    AF = mybir.ActivationFunctionType
    OP = mybir.AluOpType
    DR = mybir.MatmulPerfMode.DoubleRow

    nc = bacc.Bacc("TRN2", target_bir_lowering=False, debug=False)

    def din(name, shape, dt=bf16):
        return nc.dram_tensor(name, shape, dt, kind="ExternalInput").ap()

    x8h = din("x8h", [128, DT2 * S], f8)     # x^T tiled fp8 hi
    x8l = din("x8l", [128, DT2 * S], f8)     # x^T tiled fp8 lo
    wq8h = din("wq8h", [128, 4 * DT2 * 128], f8)   # x16-scaled
    wq8l = din("wq8l", [128, 4 * DT2 * 128], f8)
    wkv8h = din("wkv8h", [128, DT2 * 128], f8)     # k|v rows, x16-scaled
    wkv8l = din("wkv8l", [128, DT2 * 128], f8)
    wo8h = din("wo8h", [128, 4 * DIM], f8)         # x16-scaled
    wo8l = din("wo8l", [128, 4 * DIM], f8)
    bias2 = din("bias2", [128, 8], f32)      # cols 0-3: wq_b g; col 4: kv_b
    cosq = din("cosq", [128, S])             # 0.125-scaled
    sinq = din("sinq", [128, S])             # 0.125-scaled (sign in perm)
    cosk = din("cosk", [64, S])
    sinkt = din("sinkt", [64, S])
    perm = din("perm", [128, 128])           # signed rotate-half permutation
    mask01 = din("mask01", [128, 512])       # 0/1 window mask, 2 j-blocks
    sel2 = din("sel2", [128, 256])           # selA | selB
    es2 = din("es2", [128, 2], f32)          # exp(sinks), row 32*(h%4)
    out_d = nc.dram_tensor("out", [S, DIM], bf16, kind="ExternalOutput").ap()

    with tile.TileContext(nc) as tc:
        import contextlib
        with contextlib.ExitStack() as ctx:
            res = ctx.enter_context(tc.tile_pool(name="res", bufs=1))
            xh_ch = [res.tile([128, 2, S], f8, tag=f"xh{p}", name=f"xh{p}")
                     for p in range(PR)]
            xl_ch = [res.tile([128, 2, S], f8, tag=f"xl{p}", name=f"xl{p}")
                     for p in range(PR)]
            wqh_sb = res.tile([128, 4 * DT2, 128], f8, tag="wqh")
            wql_sb = res.tile([128, 4 * DT2, 128], f8, tag="wql")
            wkh_sb = res.tile([128, DT2, 128], f8, tag="wkh")
            wkl_sb = res.tile([128, DT2, 128], f8, tag="wkl")
            woh_sb = res.tile([128, 4, DIM], f8, tag="woh")
            wol_sb = res.tile([128, 4, DIM], f8, tag="wol")
            bia_sb = res.tile([128, 8], f32, tag="bia")
            cq_sb = res.tile([128, S], bf16, tag="cq")
            sq_sb = res.tile([128, S], bf16, tag="sq")
            ck_sb = res.tile([64, S], bf16, tag="ck")
            sk_sb = res.tile([64, S], bf16, tag="sk")
            pm_sb = res.tile([128, 128], bf16, tag="pm")
            mk_sb = res.tile([128, 512], bf16, tag="mk")
            sel_sb = res.tile([128, 256], bf16, tag="sel")
            es_sb = res.tile([128, 2], f32, tag="es")
            ones0 = res.tile([128, 2], bf16, tag="ones0")
            kv_sb = res.tile([128, S], bf16, tag="kv")
            kr_sb = res.tile([128, S], bf16, tag="kr")
            v_sb = [res.tile([128, 65], bf16, tag=f"v{j}", name=f"v{j}")
                    for j in range(NJ)]
            qT = [res.tile([128, S], bf16, tag=f"qT{g}", name=f"qT{g}")
                  for g in range(4)]
            qR = [res.tile([128, S], bf16, tag=f"qR{g}", name=f"qR{g}")
                  for g in range(4)]
            eT23 = [res.tile([128, 512], bf16, tag=f"e23_{h}", name=f"e23_{h}")
                    for h in range(HL)]
            atr = [res.tile([128, S], bf16, tag=f"atr{p}", name=f"atr{p}")
                   for p in range(4)]
            af8h = res.tile([128, 4, S], f8, tag="af8h")
            af8l = res.tile([128, 4, S], f8, tag="af8l")
            # dn[2*half + hg]: heads 4*hg..4*hg+3 at partitions 0/32/64/96
            dn = [res.tile([128, 512], f32, tag=f"dn{x}", name=f"dn{x}")
                  for x in range(4)]
            rdnb = [res.tile([128, 512], bf16, tag=f"rdb{x}", name=f"rdb{x}")
                    for x in range(4)]
            rscr = res.tile([128, 512], f32, tag="rscr")
            wrm = res.tile([128, 512], bf16, tag="wrm")

            # ------- resident DMAs: weights-first, x pairs streamed -------
            # SP q: wkv8, x-hi pairs, wq8h g1-g3, mirror, wo-hi+lo, vT
            # ACT q: wq8 g0, x-lo pairs, wq8l g1, tables, wq8l g2-g3
            G8 = DT2 * 128

            nc.sync.dma_start(wkh_sb[:], wkv8h[:])
            nc.sync.dma_start(wkl_sb[:], wkv8l[:])
            nc.scalar.dma_start(wqh_sb[:, 0:DT2, :], wq8h[:, 0:G8])
            nc.scalar.dma_start(wql_sb[:, 0:DT2, :], wq8l[:, 0:G8])
            for p in range(PR):
                nc.sync.dma_start(xh_ch[p][:], x8h[:, 2 * S * p:2 * S * (p + 1)])
                nc.scalar.dma_start(xl_ch[p][:], x8l[:, 2 * S * p:2 * S * (p + 1)])
            nc.sync.dma_start(wqh_sb[:, DT2:2 * DT2, :], wq8h[:, G8:2 * G8])
            nc.scalar.dma_start(wql_sb[:, DT2:2 * DT2, :], wq8l[:, G8:2 * G8])
            nc.scalar.dma_start(pm_sb[:], perm[:])
            nc.scalar.dma_start(ck_sb[:], cosk[:])
            nc.scalar.dma_start(sk_sb[:], sinkt[:])
            nc.sync.dma_start(wqh_sb[:, 2 * DT2:3 * DT2, :], wq8h[:, 2 * G8:3 * G8])
            nc.scalar.dma_start(wql_sb[:, 2 * DT2:3 * DT2, :], wq8l[:, 2 * G8:3 * G8])
            nc.scalar.dma_start(cq_sb[:], cosq[:])
            nc.scalar.dma_start(sq_sb[:], sinq[:])
            nc.sync.dma_start(wqh_sb[:, 3 * DT2:4 * DT2, :], wq8h[:, 3 * G8:4 * G8])
            nc.scalar.dma_start(wql_sb[:, 3 * DT2:4 * DT2, :], wq8l[:, 3 * G8:4 * G8])
            nc.scalar.dma_start(mk_sb[:], mask01[:])
            nc.scalar.dma_start(sel_sb[:], sel2[:])
            nc.scalar.dma_start(es_sb[:], es2[:])
            nc.scalar.dma_start(bia_sb[:], bias2[:])
            nc.scalar.dma_start(wol_sb[:], wo8l[:])
            nc.vector.memset(wrm[:], 0.0)
            nc.vector.memset(ones0[:], 1.0)
            for x in range(4):
                nc.vector.memset(dn[x][:], 1.0)

            pp = ctx.enter_context(
                tc.tile_pool(name="pp", bufs=2, space="PSUM"))
            rp = ctx.enter_context(tc.tile_pool(name="rp", bufs=2))
            eU = ctx.enter_context(tc.tile_pool(name="eU", bufs=3))
            eP = ctx.enter_context(tc.tile_pool(name="eP", bufs=6))

            _etile = {}

            def pqa(name):
                return pp.tile([128, 512], f32, tag="acc", bufs=4, name=name)

            # ------- PE warmup: spin the HAM up while inputs stream -------
            for wi in range(8):
                pw = pqa("pw")
                nc.tensor.matmul(pw[:], wrm[:, 0:128], wrm[:],
                                 start=True, stop=True)

            # ---------------- helpers ----------------
            def dr3(acc, wh, wl, p, sc, st, sp):
                """Three DoubleRow matmuls: wh*xh + wh*xl + wl*xh."""
                xh = xh_ch[p][:, :, 512 * sc:512 * (sc + 1)]
                xl = xl_ch[p][:, :, 512 * sc:512 * (sc + 1)]
                nc.tensor.matmul(acc[:], wh, xh, start=st, stop=False,
                                 perf_mode=DR)
                nc.tensor.matmul(acc[:], wh, xl, start=False, stop=False,
                                 perf_mode=DR)
                nc.tensor.matmul(acc[:], wl, xh, start=False, stop=sp,
                                 perf_mode=DR)

            def drain(dst, acc, bcol):
                nc.scalar.activation(dst, acc[:], AF.Identity,
                                     bias=bia_sb[:, bcol:bcol + 1],
                                     scale=1.0 / SCW)

            def pass1():
                """kv + q0 streamed over x pairs (DMA-paced)."""
                ak = [pqa("pak0"), pqa("pak1")]
                aq = [pqa("paq0"), pqa("paq1")]
                for p in range(PR):
                    st, sp = (p == 0), (p == PR - 1)
                    for sc in range(2):
                        dr3(ak[sc], wkh_sb[:, 2 * p:2 * p + 2, :],
                            wkl_sb[:, 2 * p:2 * p + 2, :], p, sc, st, sp)
                        dr3(aq[sc], wqh_sb[:, 2 * p:2 * p + 2, :],
                            wql_sb[:, 2 * p:2 * p + 2, :], p, sc, st, sp)
                for sc in range(2):
                    drain(kv_sb[:, 512 * sc:512 * (sc + 1)], ak[sc], 4)
                for sc in range(2):
                    drain(qT[0][:, 512 * sc:512 * (sc + 1)], aq[sc], 0)

            def sweep_g(dst, g):
                """q group g from resident x; ping-pong accs + ACT drains."""
                for sc in range(2):
                    aq = pqa("pg")
                    for p in range(PR):
                        dr3(aq, wqh_sb[:, g * DT2 + 2 * p:g * DT2 + 2 * p + 2, :],
                            wql_sb[:, g * DT2 + 2 * p:g * DT2 + 2 * p + 2, :],
                            p, sc, p == 0, p == PR - 1)
                    drain(dst[:, 512 * sc:512 * (sc + 1)], aq, g)

            def rope(dst, src, cos, sin, npart):
                """dst = src*cos + perm(src)*sin via PE perm + DVE mults."""
                for half in range(2):
                    cs = slice(512 * half, 512 * (half + 1))
                    psw = pqa("psw")
                    nc.tensor.matmul(psw[:npart, :], pm_sb[:npart, :npart],
                                     src[:npart, cs], start=True, stop=True)
                    tmp = rp.tile([128, 512], bf16, tag="tmp")
                    qc = rp.tile([128, 512], bf16, tag="qc")
                    nc.vector.tensor_tensor(tmp[:npart], psw[:npart, :],
                                            sin[:npart, cs], op=OP.mult)
                    nc.vector.tensor_tensor(qc[:npart], src[:npart, cs],
                                            cos[:npart, cs], op=OP.mult)
                    nc.vector.tensor_tensor(dst[:npart, cs], qc[:npart],
                                            tmp[:npart], op=OP.add)

            def et_tile(h, J):
                return eT23[h] if J in (2, 3) else _etile[(h, J // 2)]

            def c_head_scores(h, half):
                """scores -> exp -> mask for head h, j-blocks of `half`."""
                p, r0 = h // 2, 64 * (h % 2)
                for jp in range(2):
                    Ja = 4 * half + 2 * jp
                    ncols = 384 if Ja == 6 else 512
                    ps = pp.tile([128, 512], f32, tag="ps", bufs=2, name="ps")
                    nc.tensor.matmul(
                        ps[:, 0:256],
                        kr_sb[r0:r0 + 64, 128 * Ja:128 * (Ja + 1)],
                        qR[p][r0:r0 + 64, 128 * Ja:128 * Ja + 256],
                        start=True, stop=True)
                    nc.tensor.matmul(
                        ps[:, 256:ncols],
                        kr_sb[r0:r0 + 64, 128 * (Ja + 1):128 * (Ja + 2)],
                        qR[p][r0:r0 + 64, 128 * (Ja + 1):
                              128 * (Ja + 1) + ncols - 256],
                        start=True, stop=True)
                    eu = eU.tile([128, 512], bf16, tag="eu")
                    nc.scalar.activation(eu[:, :ncols], ps[:, :ncols], AF.Exp)
                    if Ja == 2:
                        et = eT23[h]
                    else:
                        et = eP.tile([128, 512], bf16, tag="et",
                                     name=f"et{h}_{Ja}")
                        _etile[(h, Ja // 2)] = et
                    nc.gpsimd.tensor_tensor(et[:, :ncols], eu[:, :ncols],
                                            mk_sb[:, :ncols], op=OP.mult)

            def c_head_attn(h, half):
                """attnT per i-block into PSUM; denom row; at drain."""
                p, r0 = h // 2, 64 * (h % 2)
                dr = 32 * (h % 4)
                pb = pp.tile([65, 512], f32, tag="pb", bufs=2, name="pb")
                for k in range(4):
                    I = 4 * half + k
                    if I > 0:
                        J = I - 1
                        tl = et_tile(h, J)
                        c0 = (J % 2) * 256 + 128
                        nc.tensor.matmul(
                            pb[:, 128 * k:128 * (k + 1)],
                            v_sb[J][:, 0:65], tl[:, c0:c0 + 128],
                            start=True, stop=False)
                    tl = et_tile(h, I)
                    c0 = (I % 2) * 256
                    nc.tensor.matmul(
                        pb[:, 128 * k:128 * (k + 1)],
                        v_sb[I][:, 0:65], tl[:, c0:c0 + 128],
                        start=(I == 0), stop=True)
                nc.vector.tensor_scalar_add(
                    dn[2 * half + h // 4][dr:dr + 1, :], pb[64:65, :],
                    es_sb[dr:dr + 1, (h // 4):(h // 4) + 1])
                nc.scalar.activation(atr[p][r0:r0 + 64,
                                            512 * half:512 * (half + 1)],
                                     pb[0:64, :], AF.Copy)

            def c_epilogue(half):
                with nc.allow_low_precision(reason="bf16 attn scale"):
                    for hg in range(2):
                        x = 2 * half + hg
                        nc.vector.reciprocal_approx_fast(rscr[:], dn[x][:])
                        nc.vector.tensor_copy(rdnb[x][:], rscr[:])
                    for p in range(4):
                        prt = pp.tile([128, 512], f32, tag="acc", bufs=4,
                                      name="prt")
                        nc.tensor.matmul(
                            prt[:], sel_sb[:, 128 * (p % 2):128 * (p % 2 + 1)],
                            rdnb[2 * half + p // 2][:], start=True, stop=True)
                        pc = eU.tile([128, 512], bf16, tag="prtc")
                        nc.scalar.activation(pc[:], prt[:], AF.Copy)
                        cs = slice(512 * half, 512 * (half + 1))
                        afb = rp.tile([128, 512], bf16, tag="afb")
                        nc.vector.tensor_tensor(afb[:], atr[p][:, cs],
                                                pc[:], op=OP.mult)
                        nc.vector.tensor_copy(af8h[:, p, cs], afb[:])
                        nc.vector.tensor_tensor(af8l[:, p, cs], afb[:],
                                                af8h[:, p, cs],
                                                op=OP.subtract)

            def d_block(it):
                obt = eU.tile([128, DIM], bf16, tag="ob", bufs=3, name="obt")
                ts = slice(128 * it, 128 * (it + 1))
                for dd in range(6):
                    po = pqa("po")
                    ds = slice(DDC * dd, DDC * (dd + 1))
                    for pr2 in range(2):
                        es_ = slice(2 * pr2, 2 * pr2 + 2)
                        nc.tensor.matmul(po[:, 0:DDC], af8h[:, es_, ts],
                                         woh_sb[:, es_, ds],
                                         start=(pr2 == 0), stop=False,
                                         perf_mode=DR)
                        nc.tensor.matmul(po[:, 0:DDC], af8h[:, es_, ts],
                                         wol_sb[:, es_, ds],
                                         start=False, stop=False,
                                         perf_mode=DR)
                        nc.tensor.matmul(po[:, 0:DDC], af8l[:, es_, ts],
                                         woh_sb[:, es_, ds],
                                         start=False, stop=(pr2 == 1),
                                         perf_mode=DR)
                    if dd % 2 == 0:
                        nc.scalar.activation(
                            obt[:, DDC * dd:DDC * (dd + 1)], po[:, 0:DDC],
                            AF.Copy, scale=1.0 / SCW)
                    else:
                        with nc.allow_low_precision(reason="bf16 out"):
                            nc.vector.tensor_scalar_mul(
                                obt[:, DDC * dd:DDC * (dd + 1)], po[:, 0:DDC],
                                1.0 / SCW)
                eng = nc.sync if it % 2 == 0 else nc.scalar
                eng.dma_start(out_d[128 * it:128 * (it + 1), :], obt[:])

            # ---------------- Phase A + B + C-L (interleaved) -------------
            pass1()
            rope(kr_sb, kv_sb, ck_sb, sk_sb, 64)
            nc.sync.dma_start(kr_sb[64:128, :], kr_sb[0:64, :])
            nc.sync.dma_start(woh_sb[:], wo8h[:])
            for j in range(NJ):
                nc.sync.dma_start_transpose(
                    v_sb[j][:, 0:64], kv_sb[64:128, 128 * j:128 * (j + 1)])
                nc.gpsimd.tensor_copy(v_sb[j][:, 64:65], ones0[:, 0:1])
            rope(qR[0], qT[0], cq_sb, sq_sb, 128)
            sweep_g(qT[1], 1)
            rope(qR[1], qT[1], cq_sb, sq_sb, 128)
            c_head_scores(0, 0)
            c_head_scores(1, 0)
            sweep_g(qT[2], 2)
            rope(qR[2], qT[2], cq_sb, sq_sb, 128)
            c_head_attn(0, 0)
            c_head_scores(2, 0)
            c_head_attn(1, 0)
            c_head_scores(3, 0)
            sweep_g(qT[3], 3)
            rope(qR[3], qT[3], cq_sb, sq_sb, 128)
            c_head_attn(2, 0)
            c_head_scores(4, 0)
            c_head_attn(3, 0)
            c_head_scores(5, 0)
            c_head_attn(4, 0)
            c_head_scores(6, 0)
            c_head_attn(5, 0)
            c_head_scores(7, 0)
            c_head_attn(6, 0)
            c_head_attn(7, 0)

            # ---------------- C-R interleaved with L-epi and D-left -------
            c_head_scores(0, 1)
            c_head_scores(1, 1)
            c_head_attn(0, 1)
            c_head_scores(2, 1)
            c_epilogue(0)
            c_head_attn(1, 1)
            c_head_scores(3, 1)
            d_block(0)
            c_head_attn(2, 1)
            c_head_scores(4, 1)
            c_head_attn(3, 1)
            c_head_scores(5, 1)
            d_block(1)
            c_head_attn(4, 1)
            c_head_scores(6, 1)
            c_head_attn(5, 1)
            c_head_scores(7, 1)
            d_block(2)
            c_head_attn(6, 1)
            c_head_attn(7, 1)
            c_epilogue(1)
            d_block(3)
            for it in range(4, NJ):
                d_block(it)

    nc.compile()
    return nc


def _host_prep(x, wq_w, wq_b, wk_w, wk_b, wv_w, wv_b, wo_w, wo_b, sinks):
    """Build per-core input maps (host-side sharding + fp8 layout prep)."""
    import ml_dtypes
    bf = ml_dtypes.bfloat16
    f8 = ml_dtypes.float8_e4m3
    f = np.float32

    def pair8(a):
        hi = a.astype(f8)
        lo = (a - hi.astype(f)).astype(f8)
        return hi, lo

    xT = np.ascontiguousarray(x.reshape(S, DIM).T).astype(f)   # [2880, 1024]
    xt = np.zeros((128, DT2 * S), f)
    for t in range(DT2):
        dp = min(128, max(0, DIM - 128 * t))
        if dp > 0:
            xt[:dp, S * t:S * (t + 1)] = xT[128 * t:128 * t + dp]
    x8h, x8l = pair8(xt)

    half = HD // 2
    inv_freq = 1.0 / (THETA ** (np.arange(half, dtype=np.float64) * 2.0 / HD))
    ang = np.arange(S, dtype=np.float64)[:, None] * inv_freq   # [S, 32]
    cos_t = np.cos(ang).T.astype(f)                            # [32, S]
    sin_t = np.sin(ang).T.astype(f)
    cos64 = np.concatenate([cos_t, cos_t], 0)                  # [64, S]
    sin64 = np.concatenate([sin_t, sin_t], 0)                  # sign in perm
    scale = np.float32(HD ** -0.5)
    cosq = (np.concatenate([cos64, cos64], 0) * scale).astype(bf)
    sinq = (np.concatenate([sin64, sin64], 0) * scale).astype(bf)
    cosk = cos64.astype(bf)
    sinkt = sin64.astype(bf)

    # signed rotate-half permutation, as matmul lhsT: perm[src, a] = sign
    perm = np.zeros((128, 128), f)
    for a in range(128):
        if (a // 32) % 2 == 0:
            perm[a + 32, a] = -1.0
        else:
            perm[a - 32, a] = 1.0
    perm = perm.astype(bf)

    jj = np.arange(128)[:, None]
    ii = np.arange(512)[None, :]
    ib = ii % 256
    allow_l = (jj <= ib) & (ib < 128)
    allow_r = (ib >= 128) & (jj > ib - 128)
    mask01 = np.where(allow_l | allow_r, 1.0, 0.0).astype(bf)  # [128, 512]

    sel2 = np.zeros((128, 256), f)
    for s in range(2):                       # selA: rows 0,32; selB: 64,96
        sel2[64 * s, 128 * s:128 * s + 64] = 1.0
        sel2[64 * s + 32, 128 * s + 64:128 * (s + 1)] = 1.0
    sel2 = sel2.astype(bf)

    def tile8(w):
        # w [E, DIM] -> [128, ceil(DIM/128)=DT2 tiles, E] transposed, x16
        E = w.shape[0]
        o = np.zeros((128, DT2 * E), f)
        for t in range(DT2):
            dp = min(128, max(0, DIM - 128 * t))
            if dp > 0:
                o[:dp, E * t:E * (t + 1)] = w[:, 128 * t:128 * t + dp].T
        return pair8(o * SCW)

    def esink_layout(s8):
        out = np.zeros((128, 2), f)
        for h in range(HL):
            out[32 * (h % 4), h // 4] = np.exp(np.float64(s8[h]))
        return out

    in_maps = []
    for c in range(NC):
        wq_c = wq_w[EL * c:EL * (c + 1)]                  # [512, 2880]
        wqb_c = wq_b[EL * c:EL * (c + 1)]
        wqh = np.zeros((128, 4 * DT2 * 128), f8)
        wql = np.zeros((128, 4 * DT2 * 128), f8)
        for g in range(4):
            h8, l8 = tile8(wq_c[128 * g:128 * (g + 1)])
            wqh[:, g * DT2 * 128:(g + 1) * DT2 * 128] = h8
            wql[:, g * DT2 * 128:(g + 1) * DT2 * 128] = l8
        wkv_c = np.concatenate([wk_w[HD * c:HD * (c + 1)],
                                wv_w[HD * c:HD * (c + 1)]], 0)
        wkvb_c = np.concatenate([wk_b[HD * c:HD * (c + 1)],
                                 wv_b[HD * c:HD * (c + 1)]])
        wkvh, wkvl = tile8(wkv_c)
        bias2 = np.zeros((128, 8), f)
        for g in range(4):
            bias2[:, g] = wqb_c[128 * g:128 * (g + 1)]
        bias2[:, 4] = wkvb_c
        wo_c = np.ascontiguousarray(wo_w[:, EL * c:EL * (c + 1)].T)
        wog = np.zeros((128, 4 * DIM), f)
        for et in range(4):
            wog[:, DIM * et:DIM * (et + 1)] = wo_c[128 * et:128 * (et + 1)]
        wo8h, wo8l = pair8(wog * SCW)
        in_maps.append({
            "x8h": x8h, "x8l": x8l,
            "wq8h": wqh, "wq8l": wql,
            "wkv8h": wkvh, "wkv8l": wkvl,
            "wo8h": wo8h, "wo8l": wo8l,
            "bias2": bias2,
            "cosq": cosq, "sinq": sinq, "cosk": cosk, "sinkt": sinkt,
            "perm": perm, "mask01": mask01, "sel2": sel2,
            "es2": esink_layout(sinks[HL * c:HL * (c + 1)]),
        })
    return in_maps


def run_on_hw(inputs, trace=False, **kw):
    from concourse import bass_utils
    if "nc" not in _cache:
        _cache["nc"] = _build_module()
    in_maps = _host_prep(**inputs)
    res = bass_utils.run_bass_kernel_spmd(
        _cache["nc"], in_maps, core_ids=list(range(NC)), trace=trace, **kw)
    out = np.zeros((S, DIM), np.float64)
    for c in range(NC):
        out += res.results[c]["out"].astype(np.float64)
    out = (out + inputs["wo_b"].astype(np.float64)).astype(np.float32)
    return out.reshape(B, S, DIM), res


def kernel(**inputs) -> np.ndarray:
    out, _ = run_on_hw(inputs, trace=False)
    return out


# revision 19
# speedup vs baseline: 1.3024x; 1.3024x over previous
"""Sparse (sliding-window + sink) GQA attention block on 8 TRN2 NeuronCores.

v7: full-bf16 matmul operands; streamed x with graduated per-chunk DMA
deps; weights-first two-queue schedule; PE-based v transpose; fused
256-wide attention matmuls (PSUM lazy-zero); split tail out-DMA.

Sharding: tensor-parallel over the 64 q-heads -> 8 q-heads (= 1 kv-head
group) per core; x replicated; wo partial outputs summed on host.

Per-core dataflow:
  A:  qkv projections; kv+q0 interleaved per x-tile sc-outer so ACT
      drains pipeline; x resident in SBUF (23 bf16 tiles); biases baked
      as an extra contraction row.  Warmup matmuls on a zero tile spin
      the PE clock gate up while inputs stream in.
  B:  RoPE rotate-half via a signed permutation matmul on the PE, then
      bf16 DVE mults; 0.125 score scale baked into the q-side cos/sin
      tables; v transposed on the PE against an identity rhs.
  C:  two half-passes (i<512, i>=512), software-pipelined across heads
      and interleaved under A / D.  Per head-half: scoresT into PSUM,
      exp on ACT, 0/1-mask multiply on DVE (bf16), attnT accumulated
      with fused 256-wide matmuls per j-block directly in PSUM.
      Denominators via the v ones-row; reciprocal_approx_fast; per-pair
      broadcast via a selector matmul; bf16 scale.
  D:  out[i,dd] partials (it-blocks 0-3 interleaved with C's second
      half), per-it batched bf16 DMA out alternating queues; the last
      two blocks split their DMA across both queues.
"""

import numpy as np

B, S, DIM = 1, 1024, 2880
H, HKV, HD = 64, 8, 64
GROUP = H // HKV
WINDOW = 128
THETA = 150000.0
NC = 8
HL = H // NC                 # 8 local q-heads per core
EL = HL * HD                 # 512 local q-dim
DT = (DIM + 127) // 128      # 23 d-tiles (22 full + 64)
NJ = S // 128                # 8 j/i blocks
DDC = 480                    # out-proj column chunk (6 per row-block)

_cache = {}


def _build_module():
    import concourse.bacc as bacc
    import concourse.mybir as mybir
    import concourse.tile as tile

    f32 = mybir.dt.float32
    bf16 = mybir.dt.bfloat16
    AF = mybir.ActivationFunctionType
    OP = mybir.AluOpType

    nc = bacc.Bacc("TRN2", target_bir_lowering=False, debug=False)

    def din(name, shape, dt=bf16):
        return nc.dram_tensor(name, shape, dt, kind="ExternalInput").ap()

    xt = din("xt", [128, DT * S])            # x^T tiled; tile22 row64 = 1.0
    wqg = din("wqg", [128, 4 * DT * 128])    # [(g*23+t)*128+e]; bias row64@t22
    wkvg = din("wkvg", [128, DT * 128])      # k|v; bias row64@t22
    wog = din("wog", [128, 4 * DIM])         # [2880*et + dd]
    cosq = din("cosq", [128, S])             # 0.125-scaled
    sinq = din("sinq", [128, S])             # 0.125-scaled (sign in perm)
    cosk = din("cosk", [64, S])
    sinkt = din("sinkt", [64, S])
    perm = din("perm", [128, 128])           # signed rotate-half permutation
    idm = din("idm", [128, 64])              # I64 at rows 64-127 (v transp.)
    mask01 = din("mask01", [128, 512])       # 0/1 window mask, 2 j-blocks
    sel2 = din("sel2", [128, 256])           # selA | selB
    es2 = din("es2", [128, 2], f32)          # exp(sinks), row 32*(h%4)
    out_d = nc.dram_tensor("out", [S, DIM], bf16, kind="ExternalOutput").ap()

    # x chunk boundaries (tile indices); graduated sizes, alternate queues
    XCH = [(0, 1), (1, 2), (2, 4), (4, 8), (8, 12), (12, 16), (16, 20),
           (20, 23)]

    with tile.TileContext(nc) as tc:
        import contextlib
        with contextlib.ExitStack() as ctx:
            res = ctx.enter_context(tc.tile_pool(name="res", bufs=1))
            x_ch = [res.tile([128, (b - a) * S], bf16, tag=f"xc{i}",
                             name=f"xc{i}")
                    for i, (a, b) in enumerate(XCH)]
            x_sb = []
            for i, (a, b) in enumerate(XCH):
                for t in range(a, b):
                    x_sb.append(x_ch[i][:, S * (t - a):S * (t - a + 1)])
            wq_sb = res.tile([128, 4 * DT * 128], bf16, tag="wq")
            wkv_sb = res.tile([128, DT * 128], bf16, tag="wkv")
            wo_sb = res.tile([128, 4 * DIM], bf16, tag="wo")
            cq_sb = res.tile([128, S], bf16, tag="cq")
            sq_sb = res.tile([128, S], bf16, tag="sq")
            ck_sb = res.tile([64, S], bf16, tag="ck")
            sk_sb = res.tile([64, S], bf16, tag="sk")
            pm_sb = res.tile([128, 128], bf16, tag="pm")
            id_sb = res.tile([128, 64], bf16, tag="idm")
            mk_sb = res.tile([128, 512], bf16, tag="mk")
            sel_sb = res.tile([128, 256], bf16, tag="sel")
            es_sb = res.tile([128, 2], f32, tag="es")
            ones0 = res.tile([128, 2], bf16, tag="ones0")
            kv_sb = res.tile([128, S], bf16, tag="kv")
            kr_sb = res.tile([128, S], bf16, tag="kr")
            v_sb = [res.tile([128, 65], bf16, tag=f"v{j}", name=f"v{j}")
                    for j in range(NJ)]
            qT = [res.tile([128, S], bf16, tag=f"qT{g}", name=f"qT{g}")
                  for g in range(4)]
            qR = [res.tile([128, S], bf16, tag=f"qR{g}", name=f"qR{g}")
                  for g in range(4)]
            eT23 = [res.tile([128, 512], bf16, tag=f"e23_{h}", name=f"e23_{h}")
                    for h in range(HL)]
            atr = [res.tile([128, S], bf16, tag=f"atr{p}", name=f"atr{p}")
                   for p in range(4)]
            atf = [res.tile([128, S], bf16, tag=f"atf{p}", name=f"atf{p}")
                   for p in range(4)]
            # dn[2*half + hg]: heads 4*hg..4*hg+3 at partitions 0/32/64/96
            dn = [res.tile([128, 512], f32, tag=f"dn{x}", name=f"dn{x}")
                  for x in range(4)]
            rdnb = [res.tile([128, 512], bf16, tag=f"rdb{x}", name=f"rdb{x}")
                    for x in range(4)]
            rscr = res.tile([128, 512], f32, tag="rscr")
            wrm = res.tile([128, 512], bf16, tag="wrm")

            # ------- resident DMAs: weights-first, x streamed per chunk ---
            GQ = DT * 128

            def dma_xch(eng, i):
                a, b = XCH[i]
                eng.dma_start(x_ch[i][:], xt[:, S * a:S * b])

            nc.sync.dma_start(wkv_sb[:], wkvg[:])
            nc.scalar.dma_start(wq_sb[:, 0:GQ], wqg[:, 0:GQ])
            dma_xch(nc.sync, 0)
            dma_xch(nc.scalar, 1)
            dma_xch(nc.sync, 2)
            dma_xch(nc.scalar, 3)
            dma_xch(nc.sync, 4)
            dma_xch(nc.scalar, 5)
            dma_xch(nc.sync, 6)
            dma_xch(nc.scalar, 7)
            nc.scalar.dma_start(wq_sb[:, GQ:2 * GQ], wqg[:, GQ:2 * GQ])
            nc.sync.dma_start(wq_sb[:, 3 * GQ:4 * GQ], wqg[:, 3 * GQ:4 * GQ])
            nc.scalar.dma_start(pm_sb[:], perm[:])
            nc.scalar.dma_start(id_sb[:], idm[:])
            nc.scalar.dma_start(ck_sb[:], cosk[:])
            nc.scalar.dma_start(sk_sb[:], sinkt[:])
            nc.scalar.dma_start(cq_sb[:], cosq[:])
            nc.scalar.dma_start(sq_sb[:], sinq[:])
            nc.scalar.dma_start(wq_sb[:, 2 * GQ:3 * GQ], wqg[:, 2 * GQ:3 * GQ])
            nc.scalar.dma_start(mk_sb[:], mask01[:])
            nc.scalar.dma_start(sel_sb[:], sel2[:])
            nc.scalar.dma_start(es_sb[:], es2[:])
            nc.scalar.dma_start(wo_sb[:, 2 * DIM:4 * DIM],
                                wog[:, 2 * DIM:4 * DIM])
            # (wo lo half is DMA'd on SP later, after the kr mirror)
            nc.vector.memset(wrm[:], 0.0)
            nc.vector.memset(ones0[:], 1.0)
            for x in range(4):
                nc.vector.memset(dn[x][:], 1.0)

            pp = ctx.enter_context(
                tc.tile_pool(name="pp", bufs=2, space="PSUM"))
            rp = ctx.enter_context(tc.tile_pool(name="rp", bufs=2))
            eU = ctx.enter_context(tc.tile_pool(name="eU", bufs=3))
            eP = ctx.enter_context(tc.tile_pool(name="eP", bufs=6))

            _etile = {}

            def pqt(name):
                return pp.tile([128, 512], f32, tag="pq", bufs=3, name=name)

            # ------- PE warmup: spin the HAM up while inputs stream -------
            for wi in range(13):
                pw = pqt("pw")
                nc.tensor.matmul(pw[:], wrm[:, 0:128], wrm[:],
                                 start=True, stop=True)

            # ---------------- helpers ----------------
            def proj_kv_q0():
                """kv + q0 per x-tile, sc-outer so drains pipeline."""
                for sc in range(2):
                    ak = pqt("pak")
                    aq = pqt("paq")
                    for t in range(DT):
                        dp = 128 if t < DT - 1 else DIM - 128 * (DT - 1) + 1
                        st, sp = (t == 0), (t == DT - 1)
                        nc.tensor.matmul(
                            ak[:], wkv_sb[:dp, 128 * t:128 * (t + 1)],
                            x_sb[t][:dp, 512 * sc:512 * (sc + 1)],
                            start=st, stop=sp)
                        nc.tensor.matmul(
                            aq[:], wq_sb[:dp, 128 * t:128 * (t + 1)],
                            x_sb[t][:dp, 512 * sc:512 * (sc + 1)],
                            start=st, stop=sp)
                    nc.scalar.activation(kv_sb[:, 512 * sc:512 * (sc + 1)],
                                         ak[:], AF.Copy)
                    nc.scalar.activation(qT[0][:, 512 * sc:512 * (sc + 1)],
                                         aq[:], AF.Copy)

            def v_transpose():
                """vT via PE transpose (identity rhs); append ones column."""
                for j in range(NJ):
                    pv = pp.tile([128, 64], bf16, tag="pv", bufs=1, name="pv")
                    nc.tensor.matmul(
                        pv[:], kv_sb[64:128, 128 * j:128 * (j + 1)],
                        id_sb[64:128, 0:64],
                        start=True, stop=True, is_transpose=True)
                    nc.scalar.activation(v_sb[j][:, 0:64], pv[:], AF.Copy)
                    nc.vector.tensor_copy(v_sb[j][:, 64:65], ones0[:, 0:1])

            def proj_group(dst, g):
                """2x23 matmuls (ap=512) into ping-pong psums, ACT drains."""
                for sc in range(2):
                    pq = pqt("pq")
                    for t in range(DT):
                        dp = 128 if t < DT - 1 else DIM - 128 * (DT - 1) + 1
                        nc.tensor.matmul(
                            pq[:], wq_sb[:dp, (g * DT + t) * 128:
                                         (g * DT + t + 1) * 128],
                            x_sb[t][:dp, 512 * sc:512 * (sc + 1)],
                            start=(t == 0), stop=(t == DT - 1))
                    nc.scalar.activation(dst[:, 512 * sc:512 * (sc + 1)],
                                         pq[:], AF.Copy)

            def rope(dst, src, cos, sin, npart):
                """dst = src*cos + perm(src)*sin via PE perm + DVE mults."""
                for half in range(2):
                    cs = slice(512 * half, 512 * (half + 1))
                    psw = pqt("psw")
                    nc.tensor.matmul(psw[:npart, :], pm_sb[:npart, :npart],
                                     src[:npart, cs], start=True, stop=True)
                    tmp = rp.tile([128, 512], bf16, tag="tmp")
                    qc = rp.tile([128, 512], bf16, tag="qc")
                    nc.vector.tensor_tensor(tmp[:npart], psw[:npart, :],
                                            sin[:npart, cs], op=OP.mult)
                    nc.vector.tensor_tensor(qc[:npart], src[:npart, cs],
                                            cos[:npart, cs], op=OP.mult)
                    nc.vector.tensor_tensor(dst[:npart, cs], qc[:npart],
                                            tmp[:npart], op=OP.add)

            def et_tile(h, J):
                return eT23[h] if J in (2, 3) else _etile[(h, J // 2)]

            def c_head_scores(h, half):
                """scores -> exp -> mask for head h, j-blocks of `half`."""
                p, r0 = h // 2, 64 * (h % 2)
                for jp in range(2):
                    Ja = 4 * half + 2 * jp
                    ncols = 384 if Ja == 6 else 512
                    ps = pp.tile([128, 512], f32, tag="ps", bufs=2, name="ps")
                    nc.tensor.matmul(
                        ps[:, 0:256],
                        kr_sb[r0:r0 + 64, 128 * Ja:128 * (Ja + 1)],
                        qR[p][r0:r0 + 64, 128 * Ja:128 * Ja + 256],
                        start=True, stop=True)
                    nc.tensor.matmul(
                        ps[:, 256:ncols],
                        kr_sb[r0:r0 + 64, 128 * (Ja + 1):128 * (Ja + 2)],
                        qR[p][r0:r0 + 64, 128 * (Ja + 1):
                              128 * (Ja + 1) + ncols - 256],
                        start=True, stop=True)
                    eu = eU.tile([128, 512], bf16, tag="eu")
                    nc.scalar.activation(eu[:, :ncols], ps[:, :ncols], AF.Exp)
                    if Ja == 2:
                        et = eT23[h]
                    else:
                        et = eP.tile([128, 512], bf16, tag="et",
                                     name=f"et{h}_{Ja}")
                        _etile[(h, Ja // 2)] = et
                    nc.vector.tensor_tensor(et[:, :ncols], eu[:, :ncols],
                                            mk_sb[:, :ncols], op=OP.mult)

            def c_head_attn(h, half):
                """attnT fused 256-wide per j-block into PSUM (lazy zero)."""
                p, r0 = h // 2, 64 * (h % 2)
                dr = 32 * (h % 4)
                pb = pp.tile([65, 512], f32, tag="pb", bufs=2, name="pb")
                I0 = 4 * half
                first = True
                for J in range(max(0, I0 - 1), I0 + 4):
                    tl = et_tile(h, J)
                    ec0, el = (J % 2) * 256, 256
                    lo = 128 * (J - I0)
                    if J == I0 - 1:          # right half only (i-block I0)
                        ec0, el, lo = ec0 + 128, 128, 0
                    elif J == I0 + 3:        # left half only (i-block I0+3)
                        el = 128
                    nc.tensor.matmul(
                        pb[:, lo:lo + el], v_sb[J][:, 0:65],
                        tl[:, ec0:ec0 + el],
                        start=first, stop=(J == I0 + 3),
                        skip_group_check=True)
                    first = False
                nc.vector.tensor_scalar_add(
                    dn[2 * half + h // 4][dr:dr + 1, :], pb[64:65, :],
                    es_sb[dr:dr + 1, (h // 4):(h // 4) + 1])
                nc.scalar.activation(atr[p][r0:r0 + 64,
                                            512 * half:512 * (half + 1)],
                                     pb[0:64, :], AF.Copy)

            def c_epilogue(half):
                with nc.allow_low_precision(reason="bf16 attn scale"):
                    for hg in range(2):
                        x = 2 * half + hg
                        nc.vector.reciprocal_approx_fast(rscr[:], dn[x][:])
                        nc.vector.tensor_copy(rdnb[x][:], rscr[:])
                for p in range(4):
                    prt = pp.tile([128, 512], f32, tag="ps", bufs=2,
                                  name="prt")
                    nc.tensor.matmul(
                        prt[:], sel_sb[:, 128 * (p % 2):128 * (p % 2 + 1)],
                        rdnb[2 * half + p // 2][:], start=True, stop=True)
                    pc = eU.tile([128, 512], bf16, tag="prtc")
                    nc.scalar.activation(pc[:], prt[:], AF.Copy)
                    cs = slice(512 * half, 512 * (half + 1))
                    nc.vector.tensor_tensor(atf[p][:, cs], atr[p][:, cs],
                                            pc[:], op=OP.mult)

            def d_block(it):
                obt = eU.tile([128, DIM], bf16, tag="ob", bufs=3, name="obt")
                eng = nc.sync if it % 2 == 0 else nc.scalar
                alt = nc.scalar if it % 2 == 0 else nc.sync
                for dd in range(6):
                    po = pqt("po")
                    for et in range(4):
                        nc.tensor.matmul(
                            po[:, 0:DDC],
                            atf[et][:, 128 * it:128 * (it + 1)],
                            wo_sb[:, DIM * et + DDC * dd:
                                  DIM * et + DDC * (dd + 1)],
                            start=(et == 0), stop=(et == 3))
                    if dd % 2 == 0:
                        nc.scalar.activation(
                            obt[:, DDC * dd:DDC * (dd + 1)], po[:, 0:DDC],
                            AF.Copy)
                    else:
                        nc.vector.tensor_copy(
                            obt[:, DDC * dd:DDC * (dd + 1)], po[:, 0:DDC])
                    if it >= 6 and dd == 2:
                        eng.dma_start(
                            out_d[128 * it:128 * (it + 1), 0:3 * DDC],
                            obt[:, 0:3 * DDC])
                if it >= 6:
                    alt.dma_start(
                        out_d[128 * it:128 * (it + 1), 3 * DDC:DIM],
                        obt[:, 3 * DDC:DIM])
                else:
                    eng.dma_start(out_d[128 * it:128 * (it + 1), :], obt[:])

            # ---------------- Phase A + B + C-L (interleaved) -------------
            proj_kv_q0()
            v_transpose()
            rope(kr_sb, kv_sb, ck_sb, sk_sb, 64)
            nc.sync.dma_start(kr_sb[64:128, :], kr_sb[0:64, :])
            nc.sync.dma_start(wo_sb[:, 0:2 * DIM], wog[:, 0:2 * DIM])
            rope(qR[0], qT[0], cq_sb, sq_sb, 128)
            proj_group(qT[1], 1)
            rope(qR[1], qT[1], cq_sb, sq_sb, 128)
            c_head_scores(0, 0)
            c_head_scores(1, 0)
            proj_group(qT[2], 2)
            rope(qR[2], qT[2], cq_sb, sq_sb, 128)
            c_head_attn(0, 0)
            c_head_scores(2, 0)
            c_head_attn(1, 0)
            c_head_scores(3, 0)
            proj_group(qT[3], 3)
            rope(qR[3], qT[3], cq_sb, sq_sb, 128)
            c_head_attn(2, 0)
            c_head_scores(4, 0)
            c_head_attn(3, 0)
            c_head_scores(5, 0)
            c_head_attn(4, 0)
            c_head_scores(6, 0)
            c_head_attn(5, 0)
            c_head_scores(7, 0)
            c_head_attn(6, 0)
            c_head_attn(7, 0)

            # ---------------- C-R interleaved with L-epi and D-left -------
            c_head_scores(0, 1)
            c_head_scores(1, 1)
            c_head_attn(0, 1)
            c_head_scores(2, 1)
            c_epilogue(0)
            c_head_attn(1, 1)
            c_head_scores(3, 1)
            d_block(0)
            c_head_attn(2, 1)
            c_head_scores(4, 1)
            c_head_attn(3, 1)
            c_head_scores(5, 1)
            d_block(1)
            c_head_attn(4, 1)
            c_head_scores(6, 1)
            c_head_attn(5, 1)
            c_head_scores(7, 1)
            d_block(2)
            c_head_attn(6, 1)
            c_head_attn(7, 1)
            c_epilogue(1)
            d_block(3)
            for it in range(4, NJ):
                d_block(it)

    nc.compile()
    return nc


def _host_prep(x, wq_w, wq_b, wk_w, wk_b, wv_w, wv_b, wo_w, wo_b, sinks):
    """Build per-core input maps (host-side sharding + bf16 layout prep)."""
    import ml_dtypes
    bf = ml_dtypes.bfloat16
    f = np.float32
    xT = np.ascontiguousarray(x.reshape(S, DIM).T).astype(f)   # [2880, 1024]
    xt = np.zeros((128, DT * S), f)
    for t in range(DT):
        dp = min(128, DIM - 128 * t)
        xt[:dp, S * t:S * (t + 1)] = xT[128 * t:128 * t + dp]
    xt[64, S * (DT - 1):] = 1.0                                # bias row
    xt = xt.astype(bf)

    half = HD // 2
    inv_freq = 1.0 / (THETA ** (np.arange(half, dtype=np.float64) * 2.0 / HD))
    ang = np.arange(S, dtype=np.float64)[:, None] * inv_freq   # [S, 32]
    cos_t = np.cos(ang).T.astype(f)                            # [32, S]
    sin_t = np.sin(ang).T.astype(f)
    cos64 = np.concatenate([cos_t, cos_t], 0)                  # [64, S]
    sin64 = np.concatenate([sin_t, sin_t], 0)                  # sign in perm
    scale = np.float32(HD ** -0.5)
    cosq = (np.concatenate([cos64, cos64], 0) * scale).astype(bf)
    sinq = (np.concatenate([sin64, sin64], 0) * scale).astype(bf)
    cosk = cos64.astype(bf)
    sinkt = sin64.astype(bf)

    # signed rotate-half permutation, as matmul lhsT: perm[src, a] = sign
    # out[a] = -in[a+32] for a%64<32 else +in[a-32]
    perm = np.zeros((128, 128), f)
    for a in range(128):
        if (a // 32) % 2 == 0:
            perm[a + 32, a] = -1.0
        else:
            perm[a - 32, a] = 1.0
    perm = perm.astype(bf)

    idm = np.zeros((128, 64), f)
    for i in range(64):
        idm[64 + i, i] = 1.0
    idm = idm.astype(bf)

    jj = np.arange(128)[:, None]
    ii = np.arange(512)[None, :]
    ib = ii % 256
    allow_l = (jj <= ib) & (ib < 128)
    allow_r = (ib >= 128) & (jj > ib - 128)
    mask01 = np.where(allow_l | allow_r, 1.0, 0.0).astype(bf)  # [128, 512]

    sel2 = np.zeros((128, 256), f)
    for s in range(2):                       # selA: rows 0,32; selB: 64,96
        sel2[64 * s, 128 * s:128 * s + 64] = 1.0
        sel2[64 * s + 32, 128 * s + 64:128 * (s + 1)] = 1.0
    sel2 = sel2.astype(bf)

    def tileT(w, b):
        # w [E, DIM] (+ bias b [E]) -> [128, DT*E] tiled transpose, bias@row64
        E = w.shape[0]
        o = np.zeros((128, DT * E), f)
        for t in range(DT):
            dp = min(128, DIM - 128 * t)
            o[:dp, E * t:E * (t + 1)] = w[:, 128 * t:128 * t + dp].T
        o[64, E * (DT - 1):] = b
        return o

    def esink_layout(s8):
        out = np.zeros((128, 2), f)
        for h in range(HL):
            out[32 * (h % 4), h // 4] = np.exp(np.float64(s8[h]))
        return out

    in_maps = []
    for c in range(NC):
        wq_c = wq_w[EL * c:EL * (c + 1)]                  # [512, 2880]
        wqb_c = wq_b[EL * c:EL * (c + 1)]
        wqg = np.zeros((128, 4 * DT * 128), f)
        for g in range(4):
            wqg[:, g * DT * 128:(g + 1) * DT * 128] = tileT(
                wq_c[128 * g:128 * (g + 1)], wqb_c[128 * g:128 * (g + 1)])
        wkv_c = np.concatenate([wk_w[HD * c:HD * (c + 1)],
                                wv_w[HD * c:HD * (c + 1)]], 0)
        wkvb_c = np.concatenate([wk_b[HD * c:HD * (c + 1)],
                                 wv_b[HD * c:HD * (c + 1)]])
        wo_c = np.ascontiguousarray(wo_w[:, EL * c:EL * (c + 1)].T)
        wog = np.zeros((128, 4 * DIM), f)
        for et in range(4):
            wog[:, DIM * et:DIM * (et + 1)] = wo_c[128 * et:128 * (et + 1)]
        in_maps.append({
            "xt": xt,
            "wqg": wqg.astype(bf),
            "wkvg": tileT(wkv_c, wkvb_c).astype(bf),
            "wog": wog.astype(bf),
            "cosq": cosq, "sinq": sinq, "cosk": cosk, "sinkt": sinkt,
            "perm": perm, "idm": idm, "mask01": mask01, "sel2": sel2,
            "es2": esink_layout(sinks[HL * c:HL * (c + 1)]),
        })
    return in_maps


def run_on_hw(inputs, trace=False, **kw):
    from concourse import bass_utils
    if "nc" not in _cache:
        _cache["nc"] = _build_module()
    in_maps = _host_prep(**inputs)
    res = bass_utils.run_bass_kernel_spmd(
        _cache["nc"], in_maps, core_ids=list(range(NC)), trace=trace, **kw)
    out = np.zeros((S, DIM), np.float64)
    for c in range(NC):
        out += res.results[c]["out"].astype(np.float64)
    out = (out + inputs["wo_b"].astype(np.float64)).astype(np.float32)
    return out.reshape(B, S, DIM), res


def kernel(**inputs) -> np.ndarray:
    out, _ = run_on_hw(inputs, trace=False)
    return out


# revision 23
# speedup vs baseline: 1.3620x; 1.0458x over previous
"""Sparse (sliding-window + sink) GQA attention block on 8 TRN2 NeuronCores.

v7: full-bf16 matmul operands; streamed x with graduated per-chunk DMA
deps; weights-first two-queue schedule; PE-based v transpose; fused
256-wide attention matmuls (PSUM lazy-zero); split tail out-DMA.

Sharding: tensor-parallel over the 64 q-heads -> 8 q-heads (= 1 kv-head
group) per core; x replicated; wo partial outputs summed on host.

Per-core dataflow:
  A:  qkv projections; kv+q0 interleaved per x-tile sc-outer so ACT
      drains pipeline; x resident in SBUF (23 bf16 tiles); biases baked
      as an extra contraction row.  Warmup matmuls on a zero tile spin
      the PE clock gate up while inputs stream in.
  B:  RoPE rotate-half via a signed permutation matmul on the PE, then
      bf16 DVE mults; 0.125 score scale baked into the q-side cos/sin
      tables; v transposed on the PE against an identity rhs.
  C:  two half-passes (i<512, i>=512), software-pipelined across heads
      and interleaved under A / D.  Per head-half: scoresT into PSUM,
      exp on ACT, 0/1-mask multiply on DVE (bf16), attnT accumulated
      with fused 256-wide matmuls per j-block directly in PSUM.
      Denominators via the v ones-row; reciprocal_approx_fast; per-pair
      broadcast via a selector matmul; bf16 scale.
  D:  out[i,dd] partials (it-blocks 0-3 interleaved with C's second
      half), per-it batched bf16 DMA out alternating queues; the last
      two blocks split their DMA across both queues.
"""

import numpy as np

B, S, DIM = 1, 1024, 2880
H, HKV, HD = 64, 8, 64
GROUP = H // HKV
WINDOW = 128
THETA = 150000.0
NC = 8
HL = H // NC                 # 8 local q-heads per core
EL = HL * HD                 # 512 local q-dim
DT = (DIM + 127) // 128      # 23 d-tiles (22 full + 64)
NJ = S // 128                # 8 j/i blocks
DDC = 480                    # out-proj column chunk (6 per row-block)

_cache = {}


def _build_module():
    import concourse.bacc as bacc
    import concourse.mybir as mybir
    import concourse.tile as tile

    f32 = mybir.dt.float32
    bf16 = mybir.dt.bfloat16
    AF = mybir.ActivationFunctionType
    OP = mybir.AluOpType

    nc = bacc.Bacc("TRN2", target_bir_lowering=False, debug=False)

    def din(name, shape, dt=bf16):
        return nc.dram_tensor(name, shape, dt, kind="ExternalInput").ap()

    xt = din("xt", [128, DT * S])            # x^T tiled; tile22 row64 = 1.0
    wqg = din("wqg", [128, 4 * DT * 128])    # [(g*23+t)*128+e]; bias row64@t22
    wkvg = din("wkvg", [128, DT * 128])      # k|v; bias row64@t22
    wog = din("wog", [128, 4 * DIM])         # [2880*et + dd]
    cosq = din("cosq", [128, S])             # 0.125-scaled
    sinq = din("sinq", [128, S])             # 0.125-scaled (sign in perm)
    cosk = din("cosk", [64, S])
    sinkt = din("sinkt", [64, S])
    perm = din("perm", [128, 128])           # signed rotate-half permutation
    idm = din("idm", [128, 64])              # I64 at rows 64-127 (v transp.)
    mask01 = din("mask01", [128, 512])       # 0/1 window mask, 2 j-blocks
    sel2 = din("sel2", [128, 256])           # selA | selB
    es2 = din("es2", [128, 2], f32)          # exp(sinks), row 32*(h%4)
    out_d = nc.dram_tensor("out", [S, DIM], bf16, kind="ExternalOutput").ap()

    # x chunk boundaries (tile indices); graduated sizes, alternate queues
    XCH = [(0, 1), (1, 2), (2, 4), (4, 8), (8, 12), (12, 16), (16, 20),
           (20, 23)]

    with tile.TileContext(nc) as tc:
        import contextlib
        with contextlib.ExitStack() as ctx:
            res = ctx.enter_context(tc.tile_pool(name="res", bufs=1))
            x_ch = [res.tile([128, (b - a) * S], bf16, tag=f"xc{i}",
                             name=f"xc{i}")
                    for i, (a, b) in enumerate(XCH)]
            x_sb = []
            for i, (a, b) in enumerate(XCH):
                for t in range(a, b):
                    x_sb.append(x_ch[i][:, S * (t - a):S * (t - a + 1)])
            wq_sb = res.tile([128, 4 * DT * 128], bf16, tag="wq")
            wkv_sb = res.tile([128, DT * 128], bf16, tag="wkv")
            wo_sb = res.tile([128, 4 * DIM], bf16, tag="wo")
            cq_sb = res.tile([128, S], bf16, tag="cq")
            sq_sb = res.tile([128, S], bf16, tag="sq")
            ck_sb = res.tile([64, S], bf16, tag="ck")
            sk_sb = res.tile([64, S], bf16, tag="sk")
            pm_sb = res.tile([128, 128], bf16, tag="pm")
            id_sb = res.tile([128, 64], bf16, tag="idm")
            mk_sb = res.tile([128, 512], bf16, tag="mk")
            sel_sb = res.tile([128, 256], bf16, tag="sel")
            es_sb = res.tile([128, 2], f32, tag="es")
            ones0 = res.tile([128, 2], bf16, tag="ones0")
            kv_sb = res.tile([128, S], bf16, tag="kv")
            kr_sb = res.tile([128, S], bf16, tag="kr")
            v_sb = [res.tile([128, 65], bf16, tag=f"v{j}", name=f"v{j}")
                    for j in range(NJ)]
            qT = [res.tile([128, S], bf16, tag=f"qT{g}", name=f"qT{g}")
                  for g in range(4)]
            qR = [res.tile([128, S], bf16, tag=f"qR{g}", name=f"qR{g}")
                  for g in range(4)]
            eT23 = [res.tile([128, 512], bf16, tag=f"e23_{h}", name=f"e23_{h}")
                    for h in range(HL)]
            atr = [res.tile([128, S], bf16, tag=f"atr{p}", name=f"atr{p}")
                   for p in range(4)]
            atf = [res.tile([128, S], bf16, tag=f"atf{p}", name=f"atf{p}")
                   for p in range(4)]
            # dn[2*half + hg]: heads 4*hg..4*hg+3 at partitions 0/32/64/96
            dn = [res.tile([128, 512], f32, tag=f"dn{x}", name=f"dn{x}")
                  for x in range(4)]
            rdnb = [res.tile([128, 512], bf16, tag=f"rdb{x}", name=f"rdb{x}")
                    for x in range(4)]
            rscr = res.tile([128, 512], f32, tag="rscr")
            wrm = res.tile([128, 512], bf16, tag="wrm")

            # ------- resident DMAs: weights-first, x streamed per chunk ---
            GQ = DT * 128

            def dma_xch(eng, i):
                a, b = XCH[i]
                eng.dma_start(x_ch[i][:], xt[:, S * a:S * b])

            # Only the early-needed inputs are issued up front; the rest are
            # issued mid-program, interleaved with ACT/SP work, so a DMA
            # issue never blocks the ACT compute stream at the 2-deep
            # per-queue throttle.
            nc.sync.dma_start(wkv_sb[:], wkvg[:])
            nc.scalar.dma_start(wq_sb[:, 0:GQ], wqg[:, 0:GQ])
            dma_xch(nc.sync, 0)
            dma_xch(nc.scalar, 1)
            dma_xch(nc.sync, 2)
            dma_xch(nc.scalar, 3)
            dma_xch(nc.sync, 4)
            dma_xch(nc.scalar, 5)
            dma_xch(nc.sync, 6)
            dma_xch(nc.scalar, 7)
            nc.vector.memset(wrm[:], 0.0)
            nc.vector.memset(ones0[:], 1.0)
            for x in range(4):
                nc.vector.memset(dn[x][:], 1.0)

            pp = ctx.enter_context(
                tc.tile_pool(name="pp", bufs=2, space="PSUM"))
            rp = ctx.enter_context(tc.tile_pool(name="rp", bufs=2))
            eU = ctx.enter_context(tc.tile_pool(name="eU", bufs=3))
            eP = ctx.enter_context(tc.tile_pool(name="eP", bufs=6))

            _etile = {}

            def pqt(name):
                return pp.tile([128, 512], f32, tag="pq", bufs=3, name=name)

            # ------- PE warmup: spin the HAM up while inputs stream -------
            for wi in range(13):
                pw = pqt("pw")
                nc.tensor.matmul(pw[:], wrm[:, 0:128], wrm[:],
                                 start=True, stop=True)

            # ---------------- helpers ----------------
            def proj_kv_q0_sc(sc):
                """kv + q0 over x-tiles for one sc half; ACT drains after."""
                ak = pqt("pak")
                aq = pqt("paq")
                for t in range(DT):
                    dp = 128 if t < DT - 1 else DIM - 128 * (DT - 1) + 1
                    st, sp = (t == 0), (t == DT - 1)
                    nc.tensor.matmul(
                        ak[:], wkv_sb[:dp, 128 * t:128 * (t + 1)],
                        x_sb[t][:dp, 512 * sc:512 * (sc + 1)],
                        start=st, stop=sp)
                    nc.tensor.matmul(
                        aq[:], wq_sb[:dp, 128 * t:128 * (t + 1)],
                        x_sb[t][:dp, 512 * sc:512 * (sc + 1)],
                        start=st, stop=sp)
                nc.scalar.activation(kv_sb[:, 512 * sc:512 * (sc + 1)],
                                     ak[:], AF.Copy)
                nc.scalar.activation(qT[0][:, 512 * sc:512 * (sc + 1)],
                                     aq[:], AF.Copy)

            def v_transpose():
                """vT via PE transpose (identity rhs); append ones column."""
                for j in range(NJ):
                    pv = pp.tile([128, 64], bf16, tag="pv", bufs=1, name="pv")
                    nc.tensor.matmul(
                        pv[:], kv_sb[64:128, 128 * j:128 * (j + 1)],
                        id_sb[64:128, 0:64],
                        start=True, stop=True, is_transpose=True)
                    nc.scalar.activation(v_sb[j][:, 0:64], pv[:], AF.Copy)
                    nc.vector.tensor_copy(v_sb[j][:, 64:65], ones0[:, 0:1])

            def proj_group(dst, g):
                """2x23 matmuls (ap=512) into ping-pong psums, ACT drains."""
                for sc in range(2):
                    pq = pqt("pq")
                    for t in range(DT):
                        dp = 128 if t < DT - 1 else DIM - 128 * (DT - 1) + 1
                        nc.tensor.matmul(
                            pq[:], wq_sb[:dp, (g * DT + t) * 128:
                                         (g * DT + t + 1) * 128],
                            x_sb[t][:dp, 512 * sc:512 * (sc + 1)],
                            start=(t == 0), stop=(t == DT - 1))
                    nc.scalar.activation(dst[:, 512 * sc:512 * (sc + 1)],
                                         pq[:], AF.Copy)

            def rope(dst, src, cos, sin, npart):
                """dst = src*cos + perm(src)*sin via PE perm + DVE mults."""
                for half in range(2):
                    cs = slice(512 * half, 512 * (half + 1))
                    psw = pqt("psw")
                    nc.tensor.matmul(psw[:npart, :], pm_sb[:npart, :npart],
                                     src[:npart, cs], start=True, stop=True)
                    tmp = rp.tile([128, 512], bf16, tag="tmp")
                    qc = rp.tile([128, 512], bf16, tag="qc")
                    nc.vector.tensor_tensor(tmp[:npart], psw[:npart, :],
                                            sin[:npart, cs], op=OP.mult)
                    nc.vector.tensor_tensor(qc[:npart], src[:npart, cs],
                                            cos[:npart, cs], op=OP.mult)
                    nc.vector.tensor_tensor(dst[:npart, cs], qc[:npart],
                                            tmp[:npart], op=OP.add)

            def et_tile(h, J):
                return eT23[h] if J in (2, 3) else _etile[(h, J // 2)]

            def c_head_scores(h, half):
                """scores -> exp -> mask for head h, j-blocks of `half`."""
                p, r0 = h // 2, 64 * (h % 2)
                for jp in range(2):
                    Ja = 4 * half + 2 * jp
                    ncols = 384 if Ja == 6 else 512
                    ps = pp.tile([128, 512], f32, tag="ps", bufs=2, name="ps")
                    nc.tensor.matmul(
                        ps[:, 0:256],
                        kr_sb[r0:r0 + 64, 128 * Ja:128 * (Ja + 1)],
                        qR[p][r0:r0 + 64, 128 * Ja:128 * Ja + 256],
                        start=True, stop=True)
                    nc.tensor.matmul(
                        ps[:, 256:ncols],
                        kr_sb[r0:r0 + 64, 128 * (Ja + 1):128 * (Ja + 2)],
                        qR[p][r0:r0 + 64, 128 * (Ja + 1):
                              128 * (Ja + 1) + ncols - 256],
                        start=True, stop=True)
                    eu = eU.tile([128, 512], bf16, tag="eu")
                    nc.scalar.activation(eu[:, :ncols], ps[:, :ncols], AF.Exp)
                    if Ja == 2:
                        et = eT23[h]
                    else:
                        et = eP.tile([128, 512], bf16, tag="et",
                                     name=f"et{h}_{Ja}")
                        _etile[(h, Ja // 2)] = et
                    nc.vector.tensor_tensor(et[:, :ncols], eu[:, :ncols],
                                            mk_sb[:, :ncols], op=OP.mult)

            def c_head_attn(h, half):
                """attnT fused 256-wide per j-block into PSUM (lazy zero)."""
                p, r0 = h // 2, 64 * (h % 2)
                dr = 32 * (h % 4)
                pb = pp.tile([65, 512], f32, tag="pb", bufs=2, name="pb")
                I0 = 4 * half
                first = True
                for J in range(max(0, I0 - 1), I0 + 4):
                    tl = et_tile(h, J)
                    ec0, el = (J % 2) * 256, 256
                    lo = 128 * (J - I0)
                    if J == I0 - 1:          # right half only (i-block I0)
                        ec0, el, lo = ec0 + 128, 128, 0
                    elif J == I0 + 3:        # left half only (i-block I0+3)
                        el = 128
                    nc.tensor.matmul(
                        pb[:, lo:lo + el], v_sb[J][:, 0:65],
                        tl[:, ec0:ec0 + el],
                        start=first, stop=(J == I0 + 3),
                        skip_group_check=True)
                    first = False
                nc.vector.tensor_scalar_add(
                    dn[2 * half + h // 4][dr:dr + 1, :], pb[64:65, :],
                    es_sb[dr:dr + 1, (h // 4):(h // 4) + 1])
                nc.scalar.activation(atr[p][r0:r0 + 64,
                                            512 * half:512 * (half + 1)],
                                     pb[0:64, :], AF.Copy)

            def c_epilogue(half):
                with nc.allow_low_precision(reason="bf16 attn scale"):
                    for hg in range(2):
                        x = 2 * half + hg
                        nc.vector.reciprocal_approx_fast(rscr[:], dn[x][:])
                        nc.vector.tensor_copy(rdnb[x][:], rscr[:])
                for p in range(4):
                    prt = pp.tile([128, 512], f32, tag="ps", bufs=2,
                                  name="prt")
                    nc.tensor.matmul(
                        prt[:], sel_sb[:, 128 * (p % 2):128 * (p % 2 + 1)],
                        rdnb[2 * half + p // 2][:], start=True, stop=True)
                    pc = eU.tile([128, 512], bf16, tag="prtc")
                    nc.scalar.activation(pc[:], prt[:], AF.Copy)
                    cs = slice(512 * half, 512 * (half + 1))
                    nc.vector.tensor_tensor(atf[p][:, cs], atr[p][:, cs],
                                            pc[:], op=OP.mult)

            def d_block(it):
                obt = eU.tile([128, DIM], bf16, tag="ob", bufs=3, name="obt")
                eng = nc.sync if it % 2 == 0 else nc.scalar
                alt = nc.scalar if it % 2 == 0 else nc.sync
                for dd in range(6):
                    po = pqt("po")
                    for et in range(4):
                        nc.tensor.matmul(
                            po[:, 0:DDC],
                            atf[et][:, 128 * it:128 * (it + 1)],
                            wo_sb[:, DIM * et + DDC * dd:
                                  DIM * et + DDC * (dd + 1)],
                            start=(et == 0), stop=(et == 3))
                    if dd % 2 == 0:
                        nc.scalar.activation(
                            obt[:, DDC * dd:DDC * (dd + 1)], po[:, 0:DDC],
                            AF.Copy)
                    else:
                        nc.vector.tensor_copy(
                            obt[:, DDC * dd:DDC * (dd + 1)], po[:, 0:DDC])
                    if it >= 6 and dd == 2:
                        eng.dma_start(
                            out_d[128 * it:128 * (it + 1), 0:3 * DDC],
                            obt[:, 0:3 * DDC])
                if it >= 6:
                    alt.dma_start(
                        out_d[128 * it:128 * (it + 1), 3 * DDC:DIM],
                        obt[:, 3 * DDC:DIM])
                else:
                    eng.dma_start(out_d[128 * it:128 * (it + 1), :], obt[:])

            # ---------------- Phase A + B + C-L (interleaved) -------------
            proj_kv_q0_sc(0)
            nc.scalar.dma_start(wq_sb[:, GQ:2 * GQ], wqg[:, GQ:2 * GQ])
            nc.sync.dma_start(wq_sb[:, 3 * GQ:4 * GQ], wqg[:, 3 * GQ:4 * GQ])
            proj_kv_q0_sc(1)
            nc.sync.dma_start(wq_sb[:, 2 * GQ:3 * GQ], wqg[:, 2 * GQ:3 * GQ])
            nc.sync.dma_start(pm_sb[:], perm[:])
            nc.sync.dma_start(id_sb[:], idm[:])
            nc.sync.dma_start(ck_sb[:], cosk[:])
            nc.sync.dma_start(sk_sb[:], sinkt[:])
            proj_group(qT[1], 1)
            nc.scalar.dma_start(cq_sb[:], cosq[:])
            nc.scalar.dma_start(sq_sb[:], sinq[:])
            nc.scalar.dma_start(mk_sb[:], mask01[:])
            nc.scalar.dma_start(sel_sb[:], sel2[:])
            nc.scalar.dma_start(es_sb[:], es2[:])
            nc.sync.dma_start(wo_sb[:, 0:2 * DIM], wog[:, 0:2 * DIM])
            v_transpose()
            rope(kr_sb, kv_sb, ck_sb, sk_sb, 64)
            nc.sync.dma_start(kr_sb[64:128, :], kr_sb[0:64, :])
            nc.scalar.dma_start(wo_sb[:, 2 * DIM:4 * DIM],
                                wog[:, 2 * DIM:4 * DIM])
            rope(qR[0], qT[0], cq_sb, sq_sb, 128)
            rope(qR[1], qT[1], cq_sb, sq_sb, 128)
            c_head_scores(0, 0)
            c_head_scores(1, 0)
            proj_group(qT[2], 2)
            rope(qR[2], qT[2], cq_sb, sq_sb, 128)
            c_head_attn(0, 0)
            c_head_scores(2, 0)
            c_head_attn(1, 0)
            c_head_scores(3, 0)
            proj_group(qT[3], 3)
            rope(qR[3], qT[3], cq_sb, sq_sb, 128)
            c_head_attn(2, 0)
            c_head_scores(4, 0)
            c_head_attn(3, 0)
            c_head_scores(5, 0)
            c_head_attn(4, 0)
            c_head_scores(6, 0)
            c_head_attn(5, 0)
            c_head_scores(7, 0)
            c_head_attn(6, 0)
            c_head_attn(7, 0)

            # ---------------- C-R interleaved with L-epi and D-left -------
            c_head_scores(0, 1)
            c_head_scores(1, 1)
            c_head_attn(0, 1)
            c_head_scores(2, 1)
            c_epilogue(0)
            c_head_attn(1, 1)
            c_head_scores(3, 1)
            d_block(0)
            c_head_attn(2, 1)
            c_head_scores(4, 1)
            c_head_attn(3, 1)
            c_head_scores(5, 1)
            d_block(1)
            c_head_attn(4, 1)
            c_head_scores(6, 1)
            c_head_attn(5, 1)
            c_head_scores(7, 1)
            d_block(2)
            c_head_attn(6, 1)
            c_head_attn(7, 1)
            c_epilogue(1)
            d_block(3)
            for it in range(4, NJ):
                d_block(it)

    nc.compile()
    return nc


def _host_prep(x, wq_w, wq_b, wk_w, wk_b, wv_w, wv_b, wo_w, wo_b, sinks):
    """Build per-core input maps (host-side sharding + bf16 layout prep)."""
    import ml_dtypes
    bf = ml_dtypes.bfloat16
    f = np.float32
    xT = np.ascontiguousarray(x.reshape(S, DIM).T).astype(f)   # [2880, 1024]
    xt = np.zeros((128, DT * S), f)
    for t in range(DT):
        dp = min(128, DIM - 128 * t)
        xt[:dp, S * t:S * (t + 1)] = xT[128 * t:128 * t + dp]
    xt[64, S * (DT - 1):] = 1.0                                # bias row
    xt = xt.astype(bf)

    half = HD // 2
    inv_freq = 1.0 / (THETA ** (np.arange(half, dtype=np.float64) * 2.0 / HD))
    ang = np.arange(S, dtype=np.float64)[:, None] * inv_freq   # [S, 32]
    cos_t = np.cos(ang).T.astype(f)                            # [32, S]
    sin_t = np.sin(ang).T.astype(f)
    cos64 = np.concatenate([cos_t, cos_t], 0)                  # [64, S]
    sin64 = np.concatenate([sin_t, sin_t], 0)                  # sign in perm
    scale = np.float32(HD ** -0.5)
    cosq = (np.concatenate([cos64, cos64], 0) * scale).astype(bf)
    sinq = (np.concatenate([sin64, sin64], 0) * scale).astype(bf)
    cosk = cos64.astype(bf)
    sinkt = sin64.astype(bf)

    # signed rotate-half permutation, as matmul lhsT: perm[src, a] = sign
    # out[a] = -in[a+32] for a%64<32 else +in[a-32]
    perm = np.zeros((128, 128), f)
    for a in range(128):
        if (a // 32) % 2 == 0:
            perm[a + 32, a] = -1.0
        else:
            perm[a - 32, a] = 1.0
    perm = perm.astype(bf)

    idm = np.zeros((128, 64), f)
    for i in range(64):
        idm[64 + i, i] = 1.0
    idm = idm.astype(bf)

    jj = np.arange(128)[:, None]
    ii = np.arange(512)[None, :]
    ib = ii % 256
    allow_l = (jj <= ib) & (ib < 128)
    allow_r = (ib >= 128) & (jj > ib - 128)
    mask01 = np.where(allow_l | allow_r, 1.0, 0.0).astype(bf)  # [128, 512]

    sel2 = np.zeros((128, 256), f)
    for s in range(2):                       # selA: rows 0,32; selB: 64,96
        sel2[64 * s, 128 * s:128 * s + 64] = 1.0
        sel2[64 * s + 32, 128 * s + 64:128 * (s + 1)] = 1.0
    sel2 = sel2.astype(bf)

    def tileT(w, b):
        # w [E, DIM] (+ bias b [E]) -> [128, DT*E] tiled transpose, bias@row64
        E = w.shape[0]
        o = np.zeros((128, DT * E), f)
        for t in range(DT):
            dp = min(128, DIM - 128 * t)
            o[:dp, E * t:E * (t + 1)] = w[:, 128 * t:128 * t + dp].T
        o[64, E * (DT - 1):] = b
        return o

    def esink_layout(s8):
        out = np.zeros((128, 2), f)
        for h in range(HL):
            out[32 * (h % 4), h // 4] = np.exp(np.float64(s8[h]))
        return out

    in_maps = []
    for c in range(NC):
        wq_c = wq_w[EL * c:EL * (c + 1)]                  # [512, 2880]
        wqb_c = wq_b[EL * c:EL * (c + 1)]
        wqg = np.zeros((128, 4 * DT * 128), f)
        for g in range(4):
            wqg[:, g * DT * 128:(g + 1) * DT * 128] = tileT(
                wq_c[128 * g:128 * (g + 1)], wqb_c[128 * g:128 * (g + 1)])
        wkv_c = np.concatenate([wk_w[HD * c:HD * (c + 1)],
                                wv_w[HD * c:HD * (c + 1)]], 0)
        wkvb_c = np.concatenate([wk_b[HD * c:HD * (c + 1)],
                                 wv_b[HD * c:HD * (c + 1)]])
        wo_c = np.ascontiguousarray(wo_w[:, EL * c:EL * (c + 1)].T)
        wog = np.zeros((128, 4 * DIM), f)
        for et in range(4):
            wog[:, DIM * et:DIM * (et + 1)] = wo_c[128 * et:128 * (et + 1)]
        in_maps.append({
            "xt": xt,
            "wqg": wqg.astype(bf),
            "wkvg": tileT(wkv_c, wkvb_c).astype(bf),
            "wog": wog.astype(bf),
            "cosq": cosq, "sinq": sinq, "cosk": cosk, "sinkt": sinkt,
            "perm": perm, "idm": idm, "mask01": mask01, "sel2": sel2,
            "es2": esink_layout(sinks[HL * c:HL * (c + 1)]),
        })
    return in_maps


def run_on_hw(inputs, trace=False, **kw):
    from concourse import bass_utils
    if "nc" not in _cache:
        _cache["nc"] = _build_module()
    in_maps = _host_prep(**inputs)
    res = bass_utils.run_bass_kernel_spmd(
        _cache["nc"], in_maps, core_ids=list(range(NC)), trace=trace, **kw)
    out = np.zeros((S, DIM), np.float64)
    for c in range(NC):
        out += res.results[c]["out"].astype(np.float64)
    out = (out + inputs["wo_b"].astype(np.float64)).astype(np.float32)
    return out.reshape(B, S, DIM), res


def kernel(**inputs) -> np.ndarray:
    out, _ = run_on_hw(inputs, trace=False)
    return out
